# revision 9
# baseline (speedup 1.0000x reference)
"""Trainium2 Bass kernel for BatchedAdjacency (exact Gaussian-kernel MVM).

Math per batch b (n = H*W = 4096 pixels, d = 5 guide dims, L = 16 channels):
    W[i,j]   = exp(-0.5 * ||r_i - r_j||^2)
    out[l,i] = sum_j W[i,j] * s[j,l] - s[i,l]

Distribution: 8 cores = 4 batches x 2 row-halves (2048 output pixels each).

Device algorithm per core (all matmuls fp16 with hi/lo splits, PSUM fp32):
  - MM1 computes -0.5*d2 tiles [j=128, i=512] directly via augmented features:
      lhsT rows = [r_hi(5), r_lo(5), r_hi(5), -0.5sq_hi, -0.5sq_lo, 1, 1]
      rhs  rows = [r_hi(5), r_hi(5), r_lo(5), 1, 1, -0.5sq_hi, -0.5sq_lo]
    so lhsT.T @ rhs = cross - 0.5 sq_j - 0.5 sq_i = -0.5 d2 (K = 19).
    Three j-blocks are packed into distinct 32-row PE strips (concurrent MMs)
    filling a 3-bank PSUM tile [128, 1536].
  - ScalarE applies exp() PSUM->SBUF (fp16 out) - this is the bottleneck engine.
  - MM2 accumulates out[l, i] += s_rows[j, l]^T @ W[j, i] over all 32 j-blocks
    into a persistent PSUM bank; stationary is [s_hi | s_lo] (M = 32).
  - VectorE: out = acc_hi + acc_lo - src, then DMA to HBM.
"""

import sys

if "/opt/trn_rl_repo" not in sys.path:
    sys.path.insert(0, "/opt/trn_rl_repo")

import numpy as np

import concourse.bacc as bacc
import concourse.bass as bass
import concourse.mybir as mybir
import concourse.tile as tile
from concourse.bass_utils import run_bass_kernel_spmd

BS, L, D, H, W = 4, 16, 5, 64, 64
N = H * W            # 4096 pixels
NCORES = 8
HALF = N // 2        # output pixels per core
CHUNK = 512          # i-tile (PSUM bank / fp32 matmul free-dim limit)
NCHUNK = HALF // CHUNK
JB = 128             # j-block (contraction tile)
NJB = N // JB        # 32
KAUG = 19            # augmented feature count
GS = 3               # j-blocks packed per PSUM trio tile

FP16 = mybir.dt.float16
F32 = mybir.dt.float32


def build_nc() -> bass.Bass:
    nc = bacc.Bacc()

    aug_j = nc.declare_dram_parameter("aug_j", [KAUG, N], FP16, isOutput=False)
    aug_i = nc.declare_dram_parameter("aug_i", [KAUG, HALF], FP16, isOutput=False)
    s_rows = nc.declare_dram_parameter("s_rows", [128, NJB * 48], FP16, isOutput=False)
    s_nat = nc.declare_dram_parameter("s_nat", [16, HALF], F32, isOutput=False)
    out = nc.declare_dram_parameter("out", [16, HALF], F32, isOutput=True)

    # j-block groups: trios (3 PSUM banks each), remainder duo
    groups = [list(range(g, min(g + GS, NJB))) for g in range(0, NJB, GS)]
    # pipelined emission order: (chunk, j-blocks) pairs, flattened
    sched = [(c, jbs) for c in range(NCHUNK) for jbs in groups]

    with tile.TileContext(nc) as tc:
        with (
            tc.tile_pool(name="const", bufs=1) as cpool,
            tc.tile_pool(name="wpool", bufs=3) as wpool,
            tc.tile_pool(name="ppool", bufs=2, space="PSUM") as ppool,
            tc.tile_pool(name="apool", bufs=2, space="PSUM") as apool,
            tc.tile_pool(name="opool", bufs=2) as opool,
        ):
            # replicate the 19 augmented-feature rows into the (up to four)
            # 32-partition PE strips via independent parallel DMAs
            aug_j_sb = cpool.tile([128, N], FP16)
            aug_i_sb = cpool.tile([128, HALF], FP16)
            engs = [nc.sync, nc.scalar, nc.gpsimd]
            for s in range(GS):
                engs[s % 3].dma_start(
                    out=aug_j_sb[32 * s : 32 * s + KAUG, :], in_=aug_j[:]
                )
            for s in range(GS):
                engs[(s + 1) % 3].dma_start(
                    out=aug_i_sb[32 * s : 32 * s + KAUG, :], in_=aug_i[:]
                )
            s_rows_sb = cpool.tile([128, NJB * 48], FP16)
            nc.sync.dma_start(out=s_rows_sb[:], in_=s_rows[:])
            s_nat_sb = cpool.tile([16, HALF], F32)
            nc.gpsimd.dma_start(out=s_nat_sb[:], in_=s_nat[:])

            def emit_mm1(c, jbs):
                isl = slice(c * CHUNK, (c + 1) * CHUNK)
                p = ppool.tile([128, GS * CHUNK], F32, tag="p", name="p")
                for s, jb in enumerate(jbs):
                    nc.tensor.matmul(
                        p[:, s * CHUNK : (s + 1) * CHUNK],
                        lhsT=aug_j_sb[32 * s : 32 * s + KAUG, jb * JB : (jb + 1) * JB],
                        rhs=aug_i_sb[32 * s : 32 * s + KAUG, isl],
                        start=True,
                        stop=True,
                    )
                gs = len(jbs)
                wt = wpool.tile([128, GS * CHUNK], FP16, tag="w", name="wt")
                nc.scalar.activation(
                    wt[:, : gs * CHUNK],
                    p[:, : gs * CHUNK],
                    mybir.ActivationFunctionType.Exp,
                )
                return wt

            accs = {}

            def emit_mm2(c, jbs, wt):
                if c not in accs:
                    accs[c] = apool.tile([48, CHUNK], F32, tag="acc", name="acc")
                acc = accs[c]
                for s, jb in enumerate(jbs):
                    nc.tensor.matmul(
                        acc[:],
                        lhsT=s_rows_sb[:, jb * 48 : (jb + 1) * 48],
                        rhs=wt[:, s * CHUNK : (s + 1) * CHUNK],
                        start=(jb == 0),
                        stop=(jb == NJB - 1),
                        skip_group_check=True,
                    )
                if jbs[-1] == NJB - 1:  # chunk complete: drain + store
                    isl = slice(c * CHUNK, (c + 1) * CHUNK)
                    t0 = opool.tile([16, CHUNK], F32, tag="t0", name="t0")
                    nc.vector.tensor_sub(t0[:], acc[0:16, :], s_nat_sb[:, isl])
                    o = opool.tile([16, CHUNK], F32, tag="o", name="o")
                    nc.vector.tensor_add(o[:], t0[:], acc[32:48, :])
                    nc.sync.dma_start(out=out[:, isl], in_=o[:])

            # software-pipelined emission: PE stream is MM1(G), MM1(G+1),
            # MM2(G), MM1(G+2), MM2(G+1), ... so the PE never stalls on the
            # exp of the current group before starting the next group's MM1s.
            prev = None
            for c, jbs in sched:
                wt = emit_mm1(c, jbs)
                if prev is not None:
                    emit_mm2(*prev)
                prev = (c, jbs, wt)
            emit_mm2(*prev)

    nc.finalize()
    return nc


def _hi_lo(x: np.ndarray):
    hi = x.astype(np.float16)
    lo = (x - hi.astype(np.float32)).astype(np.float16)
    return hi, lo


def prep_core_inputs(src: np.ndarray, guide: np.ndarray) -> list[dict]:
    """Shard full inputs into the 8 per-core input maps (host-side layout prep)."""
    in_maps = []
    for b in range(BS):
        refs = np.ascontiguousarray(guide[b].reshape(D, N), dtype=np.float32)
        srcs = np.ascontiguousarray(src[b].reshape(L, N), dtype=np.float32)
        sq = (refs.astype(np.float64) ** 2).sum(0)
        r_hi, r_lo = _hi_lo(refs)
        q_hi, q_lo = _hi_lo((-0.5 * sq).astype(np.float32))
        ones = np.ones((1, N), np.float16)
        augj = np.concatenate(
            [r_hi, r_lo, r_hi, q_hi[None], q_lo[None], ones, ones], axis=0
        )
        augi = np.concatenate(
            [r_hi, r_hi, r_lo, ones, ones, q_hi[None], q_lo[None]], axis=0
        )
        s_hi, s_lo = _hi_lo(srcs)
        s_rows = np.zeros((128, NJB * 48), np.float16)
        for jb in range(NJB):
            blk = slice(jb * JB, (jb + 1) * JB)
            s_rows[:, 48 * jb : 48 * jb + 16] = s_hi[:, blk].T
            s_rows[:, 48 * jb + 32 : 48 * jb + 48] = s_lo[:, blk].T
        for h in range(2):
            isl = slice(h * HALF, (h + 1) * HALF)
            in_maps.append(
                {
                    "aug_j": augj,
                    "aug_i": np.ascontiguousarray(augi[:, isl]),
                    "s_rows": s_rows,
                    "s_nat": np.ascontiguousarray(srcs[:, isl]),
                }
            )
    return in_maps


_NC_CACHE = None


def _get_nc() -> bass.Bass:
    global _NC_CACHE
    if _NC_CACHE is None:
        _NC_CACHE = build_nc()
    return _NC_CACHE


def run_on_hw(in_maps, **kwargs):
    return run_bass_kernel_spmd(_get_nc(), in_maps, core_ids=list(range(NCORES)), **kwargs)


def assemble_output(results: list[dict]) -> np.ndarray:
    out = np.empty((BS, L, N), np.float32)
    for b in range(BS):
        for h in range(2):
            out[b, :, h * HALF : (h + 1) * HALF] = results[2 * b + h]["out"]
    return out.reshape(BS, L, H, W)


def kernel(src_imgs: np.ndarray, guide_imgs: np.ndarray) -> np.ndarray:
    src = np.asarray(src_imgs, dtype=np.float32)
    guide = np.asarray(guide_imgs, dtype=np.float32)
    in_maps = prep_core_inputs(src, guide)
    res = run_on_hw(in_maps)
    return assemble_output(res.results)
